# revision 2
# baseline (speedup 1.0000x reference)
"""Pairwise cosine similarity  O = (Z/|Z_rows|) @ (Y/|Y_rows|).T  on 8 TRN2 cores.

Sharding: Z rows split across 8 cores (data parallel), Y replicated.
Each core computes a [512, 4096] block of the [4096, 4096] output.

v4 = v3 with startup pipelining:
  - Z^T is loaded as 4 k-group sub-tiles (double-buffered) interleaved with
    the first chunk's Y^T k-group sub-loads on the SP HWDGE ring, so the
    first matmuls issue ~7us in instead of waiting ~30us for both 4MB
    transposed DMAs to complete.
  - natural-layout norm copies are half-chunks (2MB) on the ACT HWDGE ring.
Everything else as v3: host-staged bf16, DMA-xbar transposed operand loads,
PE does only matmuls (268 ns/MM measured floor), norms folded at evict.
"""

import contextlib
import sys
import numpy as np

_TRN_REPO = "/opt/trn_rl_repo"
if _TRN_REPO not in sys.path:
    sys.path.insert(0, _TRN_REPO)

import ml_dtypes

import concourse.bacc as bacc
import concourse.mybir as mybir
import concourse.tile as tile
from concourse.bass_utils import run_bass_kernel_spmd
from concourse.masks import make_identity

P = 128
N_CORES = 8
F32 = mybir.dt.float32
BF16 = mybir.dt.bfloat16
KG = 4  # k-group count for zt / first-chunk yt sub-loads


def build(bz_core=512, by=4096, feat=4096, chunk=512, bench_iters=None):
    """Build + bacc-compile the SPMD program (same program on every core)."""
    assert bz_core % P == 0 and by % chunk == 0 and feat % P == 0
    m_sub = bz_core // P          # output row sub-tiles (PSUM banks)
    k_tiles = feat // P           # contraction tiles
    n_chunks = by // chunk        # Y row chunks
    j_sub = chunk // P            # row sub-tiles per Y chunk
    kg_tiles = k_tiles // KG      # k-tiles per k-group
    kg_feat = feat // KG

    nc = bacc.Bacc("TRN2", target_bir_lowering=False, debug=False,
                   num_devices=N_CORES)
    oshape = [n_chunks, P, m_sub, chunk]
    if bench_iters is None:
        z = nc.dram_tensor("zb", [bz_core, feat], BF16, kind="ExternalInput").ap()
        y = nc.dram_tensor("yb", [by, feat], BF16, kind="ExternalInput").ap()
        o = nc.dram_tensor("o", oshape, F32, kind="ExternalOutput").ap()
    else:
        z = nc.dram_tensor("zi", [bz_core, feat], BF16).ap()
        y = nc.dram_tensor("yi", [by, feat], BF16).ap()
        o = nc.dram_tensor("oi", oshape, F32).ap()
        dummy_in = nc.dram_tensor("dummy_in", [1, 64], F32,
                                  kind="ExternalInput").ap()
        dummy_out = nc.dram_tensor("dummy_out", [1, 64], F32,
                                   kind="ExternalOutput").ap()

    with tile.TileContext(nc) as tc:
        with tc.tile_pool(name="const", bufs=1) as const_pool, \
             tc.tile_pool(name="zt", bufs=2) as zt_pool, \
             tc.tile_pool(name="nat", bufs=2) as nat_pool, \
             tc.tile_pool(name="small", bufs=2) as small_pool, \
             tc.tile_pool(name="sq", bufs=1) as sq_pool, \
             tc.tile_pool(name="yt", bufs=2) as yt_pool, \
             tc.tile_pool(name="outs", bufs=2) as out_pool, \
             tc.tile_pool(name="pacc", bufs=1, space="PSUM") as pacc_pool, \
             tc.tile_pool(name="ptr", bufs=2, space="PSUM") as ptr_pool:

            ident32 = const_pool.tile([P, P], F32)
            make_identity(nc, ident32)

            if bench_iters is None:
                _loop = contextlib.nullcontext()
            else:
                _loop = tc.For_i(0, bench_iters, 1)
            with _loop:
                def row_rnorms(nat, n_sub, rdst):
                    """rdst[p, j] = 1/|row p of subtile j| for [P, n_sub, feat]."""
                    ss = small_pool.tile([P, n_sub], F32, tag="ss", name="ss")
                    for j in range(n_sub):
                        sq = sq_pool.tile([P, feat], BF16, tag="sqscratch",
                                          name="sq")
                        nc.scalar.activation(
                            sq[:], nat[:, j],
                            mybir.ActivationFunctionType.Square,
                            accum_out=ss[:, j:j + 1])
                    std = small_pool.tile([P, n_sub], F32, tag="std",
                                          name="std")
                    nc.scalar.sqrt(std[:], ss[:])
                    nc.vector.reciprocal(rdst, std[:])

                # ---- startup: interleave zt / first-chunk yt k-group loads
                # ztg[g][p, k, m] = Z[m, 128*(g*kg_tiles + k) + p]
                ztg = []
                yt0 = yt_pool.tile([P, k_tiles, chunk], BF16, tag="yt",
                                   name="yt0")
                for g in range(KG):
                    zg = zt_pool.tile([P, kg_tiles, bz_core], BF16,
                                      tag=f"ztg{g}", name="zg")
                    nc.sync.dma_start(
                        out=zg[:], in_=z[:, g * kg_feat:(g + 1) * kg_feat],
                        transpose=True)
                    nc.sync.dma_start(
                        out=yt0[:, g * kg_tiles:(g + 1) * kg_tiles, :],
                        in_=y[0:chunk, g * kg_feat:(g + 1) * kg_feat],
                        transpose=True)
                    ztg.append(zg)

                # Z natural halves for norms (ACT ring)
                rz = small_pool.tile([P, m_sub], F32, tag="rz", name="rz")
                for h in range(m_sub // 2):
                    znat = nat_pool.tile([P, 2, feat], BF16, tag="nat",
                                         name="znat")
                    nc.scalar.dma_start(
                        out=znat[:],
                        in_=z[h * 2 * P:(h + 1) * 2 * P, :].rearrange(
                            "(m p) f -> p m f", p=P))
                    row_rnorms(znat, 2, rz[:, 2 * h:2 * h + 2])

                # ---- main loop over Y chunks ----
                for c in range(n_chunks):
                    if c == 0:
                        yt = yt0
                    else:
                        yt = yt_pool.tile([P, k_tiles, chunk], BF16, tag="yt",
                                          name="yt")
                        nc.sync.dma_start(out=yt[:],
                                          in_=y[c * chunk:(c + 1) * chunk, :],
                                          transpose=True)
                    ry = small_pool.tile([P, j_sub], F32, tag="ry", name="ry")
                    for h in range(j_sub // 2):
                        ynat = nat_pool.tile([P, 2, feat], BF16, tag="nat",
                                             name="ynat")
                        nc.scalar.dma_start(
                            out=ynat[:],
                            in_=y[c * chunk + h * 2 * P:
                                  c * chunk + (h + 1) * 2 * P, :].rearrange(
                                "(j p) f -> p j f", p=P))
                        row_rnorms(ynat, 2, ry[:, 2 * h:2 * h + 2])
                    # build [1, chunk] row of 1/|y| then broadcast across parts
                    ptt = ptr_pool.tile([P, chunk], F32, tag="ptp", name="ptt")
                    for j in range(j_sub):
                        nc.tensor.transpose(ptt[:1, j * P:(j + 1) * P],
                                            ry[:, j:j + 1], ident32[:])
                    ryrow = small_pool.tile([P, chunk], F32, tag="ryrow",
                                            name="ryrow")
                    nc.vector.tensor_copy(ryrow[:1, :], ptt[:1, :])
                    ryb = small_pool.tile([P, chunk], F32, tag="ryb",
                                          name="ryb")
                    nc.gpsimd.partition_broadcast(ryb[:], ryrow[:1, :])

                    ob = out_pool.tile([P, m_sub, chunk], F32, tag="ob",
                                       name="ob")
                    for m in range(m_sub):
                        acc = pacc_pool.tile([P, chunk], F32, tag=f"acc{m}",
                                             name=f"acc{m}")
                        for k in range(k_tiles):
                            nc.tensor.matmul(
                                acc[:],
                                ztg[k // kg_tiles][:, k % kg_tiles,
                                                   m * P:(m + 1) * P],
                                yt[:, k, :],
                                start=(k == 0),
                                stop=(k == k_tiles - 1))
                        # fold 1/|z| (per-partition) on scalar engine,
                        # 1/|y| (per-column) on DVE
                        nc.scalar.mul(ob[:, m], acc[:], rz[:, m:m + 1])
                        nc.vector.tensor_mul(ob[:, m], ob[:, m], ryb[:])
                    # contiguous per-partition out-DMA (128 descriptors)
                    nc.gpsimd.dma_start(out=o[c], in_=ob[:])

            if bench_iters is not None:
                db = const_pool.tile([1, 64], F32, tag="db", name="db")
                nc.sync.dma_start(out=db[:], in_=dummy_in[:])
                nc.vector.tensor_copy(db[:], db[:])
                nc.sync.dma_start(out=dummy_out[:], in_=db[:])

    nc.compile()
    return nc


_CACHE = {}


def _get_compiled():
    if "nc" not in _CACHE:
        _CACHE["nc"] = build()
    return _CACHE["nc"]


def kernel(Z, Y):
    Zb = np.asarray(Z, dtype=np.float32).astype(ml_dtypes.bfloat16)
    Yb = np.ascontiguousarray(
        np.asarray(Y, dtype=np.float32).astype(ml_dtypes.bfloat16))
    bz = Zb.shape[0]
    shard = bz // N_CORES
    nc = _get_compiled()
    in_maps = [{"zb": np.ascontiguousarray(Zb[i * shard:(i + 1) * shard]),
                "yb": Yb}
               for i in range(N_CORES)]
    res = run_bass_kernel_spmd(nc, in_maps, list(range(N_CORES)))
    outs = []
    for i in range(N_CORES):
        op = res.results[i]["o"]  # [n_chunks, P, m_sub, chunk]
        outs.append(np.ascontiguousarray(
            np.transpose(op, (2, 1, 0, 3))).reshape(op.shape[2] * op.shape[1],
                                                    op.shape[0] * op.shape[3]))
    out = np.concatenate(outs, axis=0)
    return out


# revision 3
# speedup vs baseline: 1.1305x; 1.1305x over previous
"""Pairwise cosine similarity  O = (Z/|Z_rows|) @ (Y/|Y_rows|).T  on 8 TRN2 cores.

Sharding: Z rows split across 8 cores (data parallel), Y replicated.
Each core computes a [512, 4096] block of the [4096, 4096] output.

v7 = v4 + all Y-norm work hoisted ahead of the chunk loop:
  - Z^T is loaded as 4 k-group sub-tiles (double-buffered) interleaved with
    the first chunk's Y^T k-group sub-loads on the SP HWDGE ring, so the
    first matmuls issue ~7us in instead of waiting ~30us for both 4MB
    transposed DMAs to complete.
  - natural-layout norm copies are half-chunks (2MB) on the ACT HWDGE ring.
Everything else as v3: host-staged bf16, DMA-xbar transposed operand loads,
PE does only matmuls (268 ns/MM measured floor), norms folded at evict.
"""

import contextlib
import sys
import numpy as np

_TRN_REPO = "/opt/trn_rl_repo"
if _TRN_REPO not in sys.path:
    sys.path.insert(0, _TRN_REPO)

import ml_dtypes

import concourse.bacc as bacc
import concourse.mybir as mybir
import concourse.tile as tile
from concourse.bass_utils import run_bass_kernel_spmd
from concourse.masks import make_identity

P = 128
N_CORES = 8
F32 = mybir.dt.float32
BF16 = mybir.dt.bfloat16
KG = 4  # k-group count for zt / first-chunk yt sub-loads


def build(bz_core=512, by=4096, feat=4096, chunk=512, bench_iters=None):
    """Build + bacc-compile the SPMD program (same program on every core)."""
    assert bz_core % P == 0 and by % chunk == 0 and feat % P == 0
    m_sub = bz_core // P          # output row sub-tiles (PSUM banks)
    k_tiles = feat // P           # contraction tiles
    n_chunks = by // chunk        # Y row chunks
    j_sub = chunk // P            # row sub-tiles per Y chunk
    kg_tiles = k_tiles // KG      # k-tiles per k-group
    kg_feat = feat // KG

    nc = bacc.Bacc("TRN2", target_bir_lowering=False, debug=False,
                   num_devices=N_CORES)
    oshape = [n_chunks, P, m_sub, chunk]
    if bench_iters is None:
        z = nc.dram_tensor("zb", [bz_core, feat], BF16, kind="ExternalInput").ap()
        y = nc.dram_tensor("yb", [by, feat], BF16, kind="ExternalInput").ap()
        o = nc.dram_tensor("o", oshape, F32, kind="ExternalOutput").ap()
    else:
        z = nc.dram_tensor("zi", [bz_core, feat], BF16).ap()
        y = nc.dram_tensor("yi", [by, feat], BF16).ap()
        o = nc.dram_tensor("oi", oshape, F32).ap()
        dummy_in = nc.dram_tensor("dummy_in", [1, 64], F32,
                                  kind="ExternalInput").ap()
        dummy_out = nc.dram_tensor("dummy_out", [1, 64], F32,
                                   kind="ExternalOutput").ap()

    with tile.TileContext(nc) as tc:
        with tc.tile_pool(name="const", bufs=1) as const_pool, \
             tc.tile_pool(name="zt", bufs=2) as zt_pool, \
             tc.tile_pool(name="nat", bufs=2) as nat_pool, \
             tc.tile_pool(name="small", bufs=2) as small_pool, \
             tc.tile_pool(name="sq", bufs=1) as sq_pool, \
             tc.tile_pool(name="ryr", bufs=1) as ryr_pool, \
             tc.tile_pool(name="yt", bufs=2) as yt_pool, \
             tc.tile_pool(name="outs", bufs=2) as out_pool, \
             tc.tile_pool(name="pacc", bufs=1, space="PSUM") as pacc_pool, \
             tc.tile_pool(name="ptr", bufs=2, space="PSUM") as ptr_pool:

            ident32 = const_pool.tile([P, P], F32)
            make_identity(nc, ident32)

            if bench_iters is None:
                _loop = contextlib.nullcontext()
            else:
                _loop = tc.For_i(0, bench_iters, 1)
            with _loop:
                def row_rnorms(nat, n_sub, rdst):
                    """rdst[p, j] = 1/|row p of subtile j| for [P, n_sub, feat]."""
                    ss = small_pool.tile([P, n_sub], F32, tag="ss", name="ss")
                    for j in range(n_sub):
                        sq = sq_pool.tile([P, feat], BF16, tag="sqscratch",
                                          name="sq")
                        nc.scalar.activation(
                            sq[:], nat[:, j],
                            mybir.ActivationFunctionType.Square,
                            accum_out=ss[:, j:j + 1])
                    std = small_pool.tile([P, n_sub], F32, tag="std",
                                          name="std")
                    nc.scalar.sqrt(std[:], ss[:])
                    nc.vector.reciprocal(rdst, std[:])

                # ---- startup: interleave zt / first-chunk yt k-group loads
                # ztg[g][p, k, m] = Z[m, 128*(g*kg_tiles + k) + p]
                ztg = []
                yt0 = yt_pool.tile([P, k_tiles, chunk], BF16, tag="yt",
                                   name="yt0")
                for g in range(KG):
                    zg = zt_pool.tile([P, kg_tiles, bz_core], BF16,
                                      tag=f"ztg{g}", name="zg")
                    nc.sync.dma_start(
                        out=zg[:], in_=z[:, g * kg_feat:(g + 1) * kg_feat],
                        transpose=True)
                    nc.sync.dma_start(
                        out=yt0[:, g * kg_tiles:(g + 1) * kg_tiles, :],
                        in_=y[0:chunk, g * kg_feat:(g + 1) * kg_feat],
                        transpose=True)
                    ztg.append(zg)

                # Z natural halves for norms (ACT ring)
                rz = small_pool.tile([P, m_sub], F32, tag="rz", name="rz")
                for h in range(m_sub // 2):
                    znat = nat_pool.tile([P, 2, feat], BF16, tag="nat",
                                         name="znat")
                    nc.scalar.dma_start(
                        out=znat[:],
                        in_=z[h * 2 * P:(h + 1) * 2 * P, :].rearrange(
                            "(m p) f -> p m f", p=P))
                    row_rnorms(znat, 2, rz[:, 2 * h:2 * h + 2])

                # ---- Y-norms pass: all 1/|y| rows built up front ----
                ryrows = ryr_pool.tile([P, n_chunks, chunk], F32,
                                       tag="ryrows", name="ryrows")
                for c in range(n_chunks):
                    ry = small_pool.tile([P, j_sub], F32, tag="ry", name="ry")
                    for h in range(j_sub // 2):
                        ynat = nat_pool.tile([P, 2, feat], BF16, tag="nat",
                                             name="ynat")
                        nc.scalar.dma_start(
                            out=ynat[:],
                            in_=y[c * chunk + h * 2 * P:
                                  c * chunk + (h + 1) * 2 * P, :].rearrange(
                                "(j p) f -> p j f", p=P))
                        row_rnorms(ynat, 2, ry[:, 2 * h:2 * h + 2])
                    ptt = ptr_pool.tile([P, chunk], F32, tag="ptp", name="ptt")
                    for j in range(j_sub):
                        nc.tensor.transpose(ptt[:1, j * P:(j + 1) * P],
                                            ry[:, j:j + 1], ident32[:])
                    nc.vector.tensor_copy(ryrows[:1, c, :], ptt[:1, :])

                # ---- main loop over Y chunks: pure MM + evict ----
                for c in range(n_chunks):
                    if c == 0:
                        yt = yt0
                    else:
                        yt = yt_pool.tile([P, k_tiles, chunk], BF16, tag="yt",
                                          name="yt")
                        nc.sync.dma_start(out=yt[:],
                                          in_=y[c * chunk:(c + 1) * chunk, :],
                                          transpose=True)
                    ryb = small_pool.tile([P, chunk], F32, tag="ryb",
                                          name="ryb")
                    nc.gpsimd.partition_broadcast(ryb[:], ryrows[:1, c, :])

                    ob = out_pool.tile([P, m_sub, chunk], F32, tag="ob",
                                       name="ob")
                    for m in range(m_sub):
                        acc = pacc_pool.tile([P, chunk], F32, tag=f"acc{m}",
                                             name=f"acc{m}")
                        for k in range(k_tiles):
                            nc.tensor.matmul(
                                acc[:],
                                ztg[k // kg_tiles][:, k % kg_tiles,
                                                   m * P:(m + 1) * P],
                                yt[:, k, :],
                                start=(k == 0),
                                stop=(k == k_tiles - 1))
                        # fold 1/|z| (per-partition) on scalar engine,
                        # 1/|y| (per-column) on DVE
                        nc.scalar.mul(ob[:, m], acc[:], rz[:, m:m + 1])
                        nc.vector.tensor_mul(ob[:, m], ob[:, m], ryb[:])
                    # contiguous per-partition out-DMA (128 descriptors)
                    nc.gpsimd.dma_start(out=o[c], in_=ob[:])

            if bench_iters is not None:
                db = const_pool.tile([1, 64], F32, tag="db", name="db")
                nc.sync.dma_start(out=db[:], in_=dummy_in[:])
                nc.vector.tensor_copy(db[:], db[:])
                nc.sync.dma_start(out=dummy_out[:], in_=db[:])

    nc.compile()
    return nc


_CACHE = {}


def _get_compiled():
    if "nc" not in _CACHE:
        _CACHE["nc"] = build()
    return _CACHE["nc"]


def kernel(Z, Y):
    Zb = np.asarray(Z, dtype=np.float32).astype(ml_dtypes.bfloat16)
    Yb = np.ascontiguousarray(
        np.asarray(Y, dtype=np.float32).astype(ml_dtypes.bfloat16))
    bz = Zb.shape[0]
    shard = bz // N_CORES
    nc = _get_compiled()
    in_maps = [{"zb": np.ascontiguousarray(Zb[i * shard:(i + 1) * shard]),
                "yb": Yb}
               for i in range(N_CORES)]
    res = run_bass_kernel_spmd(nc, in_maps, list(range(N_CORES)))
    outs = []
    for i in range(N_CORES):
        op = res.results[i]["o"]  # [n_chunks, P, m_sub, chunk]
        outs.append(np.ascontiguousarray(
            np.transpose(op, (2, 1, 0, 3))).reshape(op.shape[2] * op.shape[1],
                                                    op.shape[0] * op.shape[3]))
    out = np.concatenate(outs, axis=0)
    return out
